# revision 36
# baseline (speedup 1.0000x reference)
"""Bass/Trainium2 kernel for nn_Attn_1185410973711 (additive attention scores).

Computation (reference, fp32):
    W_s = W_attn[:, :H]; W_e = W_attn[:, H:]
    energy  = tanh(output @ W_s.T [:,None,:] + einsum('bse,he->bsh', enc, W_e) + b_attn)
    scores  = einsum('bsh,h->bs', energy, v) - 1000*(mask==0)
    out     = softmax(scores, axis=-1)           # [B, 1, S]

Strategy: data-parallel over batch B=32 across 8 NeuronCores (4 batches per
core); W_attn/b_attn/v replicated.

Mask compaction: positions with encoder_mask==0 receive a -1000 penalty, and
exp(-1000-ish) underflows to exactly 0.0 in fp32, so masked positions
contribute nothing to the softmax numerator or denominator. The host gathers
only the unmasked encoder columns per batch (~50% with this input
distribution), pads each batch to a whole number of 64-column half-tiles
(padded columns carry the -1000 penalty themselves, so they also produce
exact zeros), runs the kernel on the compacted sequence, and scatters
results back into a zero [B, 1, S] output. This nearly halves the dominant
matmul.

Slot-sorted assignment: the SPMD program is shared by all 8 cores, so the
per-slot length must only match across cores, not across the 4 batch slots.
Batches are sorted by unmasked count and rank-grouped into slots sized in
64-column halves, so short batches don't get padded to the global worst
case; at most BPC-1 boundary tiles mix two batches, handled by split c_rep
adds, split exp accumulation, and per-half normalization scalars.

The compacted encoder block (bf16, pre-transposed to [e, s] on the host) is
DMA'd fully into SBUF in the prologue; the steady-state loop runs with zero
input DMA. The enc_proj matmul keeps enc tiles stationary / W_e moving so
PSUM lands as [s_part, h_free]; the v-dot leaves the PE (DVE multiply +
scalar-engine accumulate along the free axis, with the mask penalty folded
into the accumulate as a per-partition bias of penalty/H). The softmax tail
is a single drain: ONE PE transpose of the whole [128, sum(NT)] score block,
one exp with fused row-sums, and per-batch totals/broadcasts via two small
segment-indicator matmuls, so no PE stall ever interrupts the matmul
stream.
"""

import contextlib
import math

import numpy as np

B, S, H = 32, 2048, 512
E2 = 2 * H            # 1024, encoder feature dim
N_CORES = 8
BPC = B // N_CORES    # 4 batches per core
NK = E2 // 128        # 8 contraction tiles


def _split_drain_context(nc):
    """TileContext subclass working around a walrus limit in this build: the
    kernel-tail drain rejects instructions carrying more than one semaphore
    wait. See enforce_wait_limit()."""
    import concourse.tile as tile
    from concourse.vector_clock import ScopedClock

    class TileContextSplitDrain(tile.TileContext):
        def _drain_and_barrier(self, tick_clock, wait_clock):
            probe = self.nc.sync.nop(nofuse=True, hint="tail_wait_probe")
            wait_clock.add_sem_waits(
                probe.ins, ScopedClock({None: tick_clock.global_clock})
            )
            si = probe.ins.sync_info
            waits = list(si.on_wait or []) if si is not None else []
            if si is not None:
                si.on_wait.clear()
            by_name = {h.name: h for h in self.sems.allocated().values()}
            for w in waits:
                h = by_name.get(w.ant_name)
                assert h is not None, f"missing semaphore handle for {w.ant_name}"
                self.nc.sync.wait_ge(h, w.wait_value)
            self.nc.sync.drain()
            self.nc.all_engine_barrier()
            popped = self.nc._tile_sem_poison_stack.pop()
            assert popped is self._sem_poison
            self.nc.clear_and_free_semaphores(list(self.sems.allocated().values()))
            self.nc.all_engine_barrier()

    return TileContextSplitDrain(nc)


def enforce_wait_limit(nc, limit=1):
    """Hoist excess semaphore waits onto inserted same-engine event-sem wait
    instructions placed immediately before the over-budget instruction.
    In-order engine execution makes an earlier wait strictly conservative,
    so this is always sound. Several opcodes in this walrus build (notably
    self-loading fp32 matmuls and Drain) reject multi-wait encodings."""
    import copy

    template = None
    for fn in nc.m.functions:
        for bb in fn.blocks:
            for ins in bb.instructions:
                if type(ins).__name__ == "InstEventSemaphore":
                    si = ins.sync_info
                    if si and si.on_wait and len(si.on_wait) == 1:
                        template = ins
                        break
            if template:
                break
        if template:
            break

    n_new = 0
    for fn in nc.m.functions:
        for bb in fn.blocks:
            il = bb.instructions
            new_il = []
            changed = False
            for ins in il:
                si = ins.sync_info
                waits = list(si.on_wait) if si and si.on_wait else []
                if len(waits) > limit and type(ins).__name__ != "InstEventSemaphore":
                    assert template is not None, "no event-sem template found"
                    for w in waits[limit:]:
                        c = copy.deepcopy(template)
                        n_new += 1
                        c.name = f"I-waitfix-{n_new}"
                        c.engine = ins.engine
                        csi = c.sync_info
                        csi.on_wait.clear()
                        csi.on_wait.append(w)
                        csi.on_update.clear()
                        new_il.append(c)
                    si.on_wait.clear()
                    for w in waits[:limit]:
                        si.on_wait.append(w)
                    changed = True
                new_il.append(ins)
            if changed:
                il[:] = new_il
    return n_new


def _plan(encoder_mask):
    """Slot-sorted batch assignment at 64-column granularity. Returns
    (order, counts, L) where order[8*j + c] is the original batch index
    handled by core c, slot j; L[j] is the per-slot length in 64-column
    half-tiles (max over cores), so slots pack into ceil(sum(L)/2) full
    128-column tiles with at most BPC-1 mixed boundary tiles."""
    counts = np.asarray(encoder_mask != 0).sum(axis=1).astype(int)
    order = np.argsort(-counts, kind="stable")
    L = []
    for j in range(BPC):
        grp = counts[order[j * N_CORES:(j + 1) * N_CORES]]
        L.append(max(1, int(math.ceil(int(grp.max()) / 64))))
    return order, counts, L


def _geometry(L):
    """Derived packing geometry from per-slot half-tile lengths."""
    NH = sum(L)
    NTS = (NH + 1) // 2
    Hoff = [sum(L[:j]) for j in range(BPC)]
    soh = []
    for j in range(BPC):
        soh += [j] * L[j]
    soh += [BPC - 1] * (2 * NTS - NH)     # padding half if NH is odd
    s0 = [soh[2 * t] for t in range(NTS)]
    s1 = [soh[2 * t + 1] for t in range(NTS)]
    return NH, NTS, Hoff, s0, s1


def build_nc(reps=1, plan=(17, 17, 16, 16)):
    """Build the per-core Bass program: per-slot compacted sequences of
    plan[j]*64 columns concatenated along s (boundary 128-tiles may mix two
    slots). reps>1 wraps the steady-state body in a For_i loop re-running
    the identical computation (for timing)."""
    import concourse.bass as bass
    from concourse import mybir

    f32 = mybir.dt.float32
    bf16 = mybir.dt.bfloat16
    Tanh = mybir.ActivationFunctionType.Tanh
    Exp = mybir.ActivationFunctionType.Exp
    Ident = mybir.ActivationFunctionType.Identity

    L = list(plan)
    NH, NTS, Hoff, s0, s1 = _geometry(L)
    SC = NTS * 128                       # flat compacted columns per core

    nc = bass.Bass("TRN2", target_bir_lowering=False, debug=False)

    encC_d = nc.dram_tensor("encC", [NK, 128, SC], bf16, kind="ExternalInput")
    weT_d = nc.dram_tensor("weT", [2 * H, H], bf16, kind="ExternalInput")
    wsT_d = nc.dram_tensor("wsT", [H, H], bf16, kind="ExternalInput")
    outB_d = nc.dram_tensor("outB", [BPC, 4, 128, 128], bf16, kind="ExternalInput")
    bAR_d = nc.dram_tensor("bAR", [128, H], f32, kind="ExternalInput")
    vR_d = nc.dram_tensor("vR", [128, H], bf16, kind="ExternalInput")
    mkC_d = nc.dram_tensor("mkC", [NTS, 128], f32, kind="ExternalInput")
    # segment indicators (see _shard_inputs for definitions)
    segA_d = nc.dram_tensor("segA", [NTS, BPC], f32, kind="ExternalInput")
    segE_d = nc.dram_tensor("segE", [NTS, BPC], f32, kind="ExternalInput")
    segD0_d = nc.dram_tensor("segD0", [BPC, NTS], f32, kind="ExternalInput")
    segD1_d = nc.dram_tensor("segD1", [BPC, NTS], f32, kind="ExternalInput")
    eye_d = nc.dram_tensor("eye", [128, 128], f32, kind="ExternalInput")
    out_d = nc.dram_tensor("out", [NTS, 128], f32, kind="ExternalOutput")

    tc = _split_drain_context(nc)
    with tc:
        with contextlib.ExitStack() as ctx:
            const = ctx.enter_context(tc.tile_pool(name="const", bufs=1))
            prep = ctx.enter_context(tc.tile_pool(name="prep", bufs=6))
            enrg = ctx.enter_context(tc.tile_pool(name="enrg", bufs=6))
            scrp = ctx.enter_context(tc.tile_pool(name="scrp", bufs=4))
            rowp = ctx.enter_context(tc.tile_pool(name="rowp", bufs=1))
            pe_p = ctx.enter_context(tc.tile_pool(name="pe_p", bufs=7, space="PSUM"))
            ms_p = ctx.enter_context(tc.tile_pool(name="ms_p", bufs=1, space="PSUM"))

            enc_sb = const.tile([128, NK, SC], bf16)
            we_sb = const.tile([128, NK, H], bf16)        # W_e.T tiles [e,k,h]
            ws_sb = const.tile([128, H // 128, H], bf16)  # W_s.T tiles
            ob_sb = const.tile([128, BPC, H // 128, 128], bf16)  # output bcast
            bAR_sb = const.tile([128, H], f32)
            vR_sb = const.tile([128, H], bf16)
            mk_sb = const.tile([128, NTS], f32)
            segA_sb = const.tile([NTS, BPC], f32)
            segE_sb = const.tile([NTS, BPC], f32)
            segD0_sb = const.tile([BPC, NTS], f32)
            segD1_sb = const.tile([BPC, NTS], f32)
            eye_sb = const.tile([128, 128], f32)

            nc.sync.dma_start(we_sb[:], weT_d.ap().rearrange("(k p) h -> p k h", p=128))
            nc.sync.dma_start(
                ws_sb[:], wsT_d.ap().rearrange("(k p) h -> p k h", p=128)
            )
            nc.sync.dma_start(
                ob_sb[:], outB_d.ap().rearrange("b k p m -> p b k m")
            )
            nc.sync.dma_start(bAR_sb[:], bAR_d.ap()[:])
            nc.sync.dma_start(vR_sb[:], vR_d.ap()[:])
            nc.sync.dma_start(mk_sb[:], mkC_d.ap().rearrange("t p -> p t"))
            nc.sync.dma_start(segA_sb[:], segA_d.ap()[:])
            nc.sync.dma_start(segE_sb[:], segE_d.ap()[:])
            nc.sync.dma_start(segD0_sb[:], segD0_d.ap()[:])
            nc.sync.dma_start(segD1_sb[:], segD1_d.ap()[:])
            nc.sync.dma_start(eye_sb[:], eye_d.ap()[:])
            # compacted encoder block, resident for the whole kernel
            for k in range(NK):
                nc.sync.dma_start(
                    enc_sb[:, k, :], encC_d.ap()[k]
                )

            # ---- c_rep[j] = broadcast(output[j] @ W_s.T + b_attn) ---------
            # outB is output[j] replicated along M on the host, so the state
            # matmul directly yields the row-broadcast [128, H] result; also
            # serves as the PE warm-up burst during the enc DMA.
            c_rep = const.tile([128, BPC, H], f32)
            for j in range(BPC):
                pc = ms_p.tile([128, H], f32, tag="misc", name=f"pc{j}")
                for k in range(H // 128):
                    nc.tensor.matmul(
                        pc[:],
                        ob_sb[:, j, k, :],
                        ws_sb[:, k, :],
                        start=(k == 0),
                        stop=(k == H // 128 - 1),
                    )
                nc.vector.tensor_add(c_rep[:, j, :], pc[:], bAR_sb[:])

            # ping-pong score buffers for the unrolled software pipeline
            UNROLL = 8
            scbufs = [
                rowp.tile([128, NTS], f32, tag=f"sccols{u}", name=f"sccols{u}")
                for u in range(UNROLL)
            ]

            def elementwise(sc, ps, t):
                pre = prep.tile([128, H], f32, tag="pre")
                if s0[t] == s1[t]:
                    nc.vector.tensor_add(pre[:], ps[:], c_rep[:, s0[t], :])
                else:
                    # boundary tile: the two 64-row halves belong to
                    # different batches and need different c_rep vectors
                    nc.vector.tensor_add(
                        pre[0:64, :], ps[0:64, :], c_rep[0:64, s0[t], :]
                    )
                    nc.vector.tensor_add(
                        pre[64:128, :], ps[64:128, :],
                        c_rep[64:128, s1[t], :],
                    )
                en = enrg.tile([128, H], bf16, tag="en")
                nc.scalar.activation(en[:], pre[:], Tanh)
                scr = scrp.tile([128, H], bf16, tag="scr")
                nc.vector.tensor_mul(scr[:], en[:], vR_sb[:])
                dmp = scrp.tile([128, H], bf16, tag="dmp")
                # bias = mask_penalty/H folded into the H-element accum:
                # accum = sum_h(scr_h + mk/H) = score + mask_penalty
                nc.scalar.activation(
                    dmp[:], scr[:], Ident,
                    bias=mk_sb[:, t:t + 1],
                    accum_out=sc[:, t:t + 1],
                )

            def tile_loop(sc, t0=0, t1=NTS):
                # pairwise-interleaved psum groups: alternating two banks
                # matmul-by-matmul hides the accumulation-group transition
                # bubble at each start/stop boundary
                t = t0
                while t < t1:
                    if t + 1 < t1:
                        psa = pe_p.tile([128, H], f32, tag="pe")
                        psb = pe_p.tile([128, H], f32, tag="pe")
                        for k in range(NK):
                            nc.tensor.matmul(
                                psa[:],
                                enc_sb[:, k, t * 128:(t + 1) * 128],
                                we_sb[:, k, :],
                                start=(k == 0),
                                stop=(k == NK - 1),
                            )
                            nc.tensor.matmul(
                                psb[:],
                                enc_sb[:, k, (t + 1) * 128:(t + 2) * 128],
                                we_sb[:, k, :],
                                start=(k == 0),
                                stop=(k == NK - 1),
                            )
                        elementwise(sc, psa, t)
                        elementwise(sc, psb, t + 1)
                        t += 2
                    else:
                        ps = pe_p.tile([128, H], f32, tag="pe")
                        for k in range(NK):
                            nc.tensor.matmul(
                                ps[:],
                                enc_sb[:, k, t * 128:(t + 1) * 128],
                                we_sb[:, k, :],
                                start=(k == 0),
                                stop=(k == NK - 1),
                            )
                        elementwise(sc, ps, t)
                        t += 1

            # Softmax drain, consuming a completed score buffer (in the
            # timed loop: the PREVIOUS computation's, overlapping the
            # current computation's matmul stream). Split so the PE-queue
            # transpose/tot/rb instructions are emitted a few tile groups
            # in, after their scalar/DVE deps (exp -> reciprocal) have had
            # time to complete -- otherwise the PE would stall at the
            # stage boundaries.
            dstate = {}

            def drain_head(sc):
                dstate["expv"] = rowp.tile(
                    [NTS, 128], f32, tag="expv", name="expv"
                )
                dstate["accH"] = rowp.tile(
                    [NTS, 2], f32, tag="accH", name="accH"
                )
                expv, accH = dstate["expv"], dstate["accH"]
                tp = ms_p.tile([NTS, 128], f32, tag="misc", name="tp")
                nc.tensor.transpose(tp[:], sc[:], eye_sb[:])
                # uniform per-half exp sums: every tile's two 64-column
                # halves accumulate separately (free-range splits only; PSUM
                # partition offsets must be 32-aligned so per-mixed-tile row
                # slices are not expressible). Pure tiles simply have both
                # halves assigned to the same slot in segA/segE.
                nc.scalar.activation(
                    expv[:, 0:64], tp[:, 0:64], Exp,
                    accum_out=accH[:, 0:1],
                )
                nc.scalar.activation(
                    expv[:, 64:128], tp[:, 64:128], Exp,
                    accum_out=accH[:, 1:2],
                )

            def drain_mid():
                accH = dstate["accH"]
                tot = ms_p.tile([BPC, 1], f32, tag="misc", name="tot")
                nc.tensor.matmul(
                    tot[:], segA_sb[:], accH[:, 0:1], start=True, stop=False
                )
                nc.tensor.matmul(
                    tot[:], segE_sb[:], accH[:, 1:2], start=False, stop=True
                )
                rec = rowp.tile([BPC, 1], f32, tag="rec", name="rec")
                nc.vector.reciprocal(rec[:], tot[:])
                dstate["rec"] = rec

            def drain_tail():
                expv = dstate["expv"]
                outv = rowp.tile([NTS, 128], f32, tag="outv")
                rbH0 = ms_p.tile([NTS, 1], f32, tag="misc", name="rbH0")
                nc.tensor.matmul(
                    rbH0[:], segD0_sb[:], dstate["rec"][:], start=True, stop=True
                )
                rbH1 = ms_p.tile([NTS, 1], f32, tag="misc", name="rbH1")
                nc.tensor.matmul(
                    rbH1[:], segD1_sb[:], dstate["rec"][:], start=True, stop=True
                )
                rc2 = rowp.tile([NTS, 2], f32, tag="rc2", name="rc2")
                nc.vector.tensor_copy(rc2[:, 0:1], rbH0[:])
                nc.vector.tensor_copy(rc2[:, 1:2], rbH1[:])
                nc.vector.tensor_scalar_mul(
                    outv[:, 0:64], expv[:, 0:64], rc2[:, 0:1]
                )
                nc.vector.tensor_scalar_mul(
                    outv[:, 64:128], expv[:, 64:128], rc2[:, 1:2]
                )
                nc.sync.dma_start(out_d.ap()[:], outv[:])

            def compute_and_drain_prev(sc_cur, sc_prev):
                # one computation into sc_cur with the drain of sc_prev
                # interleaved into the first tile groups
                b1, b2, b3 = min(2, NTS), min(4, NTS), min(6, NTS)
                tile_loop(sc_cur, 0, b1)
                drain_head(sc_prev)
                tile_loop(sc_cur, b1, b2)
                drain_mid()
                tile_loop(sc_cur, b2, b3)
                drain_tail()
                tile_loop(sc_cur, b3, NTS)

            if reps == 1:
                tile_loop(scbufs[0])
                drain_head(scbufs[0])
                drain_mid()
                drain_tail()
            else:
                from concourse import mybir as _mb

                # software pipeline, unrolled: each computation drains the
                # previous computation's scores while streaming its own
                # matmuls, and the For_i barrier + last elementwise tail
                # amortize over UNROLL computations per iteration. Fill one
                # computation before the loop, drain the final one after;
                # the loop runs (reps-1)/UNROLL iterations of UNROLL
                # computations, so delta/(reps-1) in test.py measures
                # exactly one overlapped computation.
                U = UNROLL if (reps - 1) % UNROLL == 0 else 1
                tile_loop(scbufs[U - 1])
                with tc.For_i(
                    0, (reps - 1) // U, 1,
                    hint_engines=(
                        _mb.EngineType.PE, _mb.EngineType.Activation,
                        _mb.EngineType.SP, _mb.EngineType.DVE,
                    ),
                ):
                    for u in range(U):
                        compute_and_drain_prev(scbufs[u], scbufs[(u - 1) % U])
                drain_head(scbufs[U - 1])
                drain_mid()
                drain_tail()

    enforce_wait_limit(nc)
    return nc


def _shard_inputs(output, encoder_outputs, encoder_mask, W_attn, b_attn, v):
    """Returns (in_maps, meta) where meta = (order, counts, L) describes
    the slot-sorted batch assignment for the output scatter."""
    import ml_dtypes

    order, counts, L = _plan(encoder_mask)
    NH, NTS, Hoff, s0, s1 = _geometry(L)
    SC = NTS * 128

    wT32 = np.ascontiguousarray(W_attn.T.astype(np.float32))        # [1536, 512]
    weT = wT32[H:].astype(ml_dtypes.bfloat16)                       # [1024, 512]
    wsT = wT32[:H].astype(ml_dtypes.bfloat16)                       # [512, 512]
    eye = np.eye(128, dtype=np.float32)
    bAR = np.broadcast_to(b_attn.astype(np.float32), (128, H)).copy()
    vR = np.broadcast_to(
        v.astype(np.float32).astype(ml_dtypes.bfloat16), (128, H)
    ).copy()
    # segment indicators over tiles:
    #   segA[t, b] / segE[t, b]: half 0 / half 1 of tile t belongs to slot b
    #   segD0/segD1: the same indicators transposed, used to broadcast
    #   per-slot reciprocals back to tile halves
    segA = np.zeros((NTS, BPC), dtype=np.float32)
    segE = np.zeros((NTS, BPC), dtype=np.float32)
    for t in range(NTS):
        segA[t, s0[t]] = 1.0
        segE[t, s1[t]] = 1.0
    segD0 = np.ascontiguousarray(segA.T)
    segD1 = np.ascontiguousarray(segE.T)

    idx_list = [np.nonzero(encoder_mask[b] != 0)[0] for b in range(B)]

    in_maps = []
    for c in range(N_CORES):
        encC = np.zeros((SC, E2), dtype=ml_dtypes.bfloat16)
        # penalty is pre-divided by H: the kernel folds it into the
        # H-element accumulate as a per-partition bias (H adds of mk/H).
        # -1000/512 = -1.953125 is exact in fp32.
        mkC = np.full((SC,), -1000.0 / H, dtype=np.float32)
        outs = np.empty((BPC, H), dtype=np.float32)
        for j in range(BPC):
            gb = int(order[j * N_CORES + c])
            ix = idx_list[gb]
            c0 = Hoff[j] * 64
            encC[c0:c0 + len(ix)] = encoder_outputs[gb, ix].astype(
                ml_dtypes.bfloat16
            )
            mkC[c0:c0 + len(ix)] = 0.0
            outs[j] = output[gb].astype(np.float32)
        # transpose to [e, s] layout for contraction-ready DMA
        encT = np.ascontiguousarray(encC.T).reshape(NK, 128, SC)
        outB = np.broadcast_to(
            outs.astype(ml_dtypes.bfloat16).reshape(BPC, 4, 128, 1),
            (BPC, 4, 128, 128),
        ).copy()
        in_maps.append({
            "encC": encT, "weT": weT, "wsT": wsT, "outB": outB,
            "bAR": bAR, "vR": vR, "mkC": mkC.reshape(NTS, 128),
            "segA": segA, "segE": segE, "segD0": segD0, "segD1": segD1,
            "eye": eye,
        })
    return in_maps, (order, counts, L, idx_list)


def kernel(output, encoder_outputs, encoder_mask, W_attn, b_attn, v):
    from concourse.bass_utils import run_bass_kernel_spmd

    output = np.asarray(output)
    encoder_outputs = np.asarray(encoder_outputs)
    encoder_mask = np.asarray(encoder_mask)
    W_attn = np.asarray(W_attn)
    b_attn = np.asarray(b_attn)
    v = np.asarray(v)

    in_maps, (order, counts, L, idx_list) = _shard_inputs(
        output, encoder_outputs, encoder_mask, W_attn, b_attn, v
    )
    NH, NTS, Hoff, s0, s1 = _geometry(L)
    nc = build_nc(plan=L)
    res = run_bass_kernel_spmd(nc, in_maps, core_ids=list(range(N_CORES)))
    full = np.zeros((B, S), dtype=np.float32)
    for j in range(BPC):
        for c in range(N_CORES):
            gb = int(order[j * N_CORES + c])
            cnt, ix = int(counts[gb]), idx_list[gb]
            if cnt == 0:
                # all positions masked: softmax over uniform -1000 scores
                full[gb] = 1.0 / S
                continue
            flat = res.results[c]["out"].reshape(-1)
            c0 = Hoff[j] * 64
            full[gb, ix] = flat[c0:c0 + cnt]
    return full.reshape(B, 1, S)
